# revision 20
# baseline (speedup 1.0000x reference)
"""Raw-bacc (no Tile) LogEncoder kernel.

Structure (single core, replicated SPMD over 8 cores; no nc.Block(), so no
entry branches / exit branches / end-of-program barrier):
  - SP: two HWDGE input DMAs of a host-packed pair ([33,32] each):
      xp = [x.T ; ones]   (lhsT; rows 0-31 get the frac chain)
      wp = [W.T ; 32*b]   (rhs; ones row turns bias into a K-row)
    The chain/PE wait on the DMAs' completion semaphores (then_inc). That
    costs SEM_PROP_DMA = 900ns of modeled signaling latency, but it is the
    only input ordering guaranteed on real HW: a drain-as-fence variant
    that started the chain ~1.5us earlier in the model was INTERMITTENTLY
    wrong on hardware (SP drains do not reliably cover in-flight DMA
    transfers). Engine-pipeline drains (DVE/PE below) are a different,
    reliable mechanism: an engine fences its own pipeline.
  - DVE chain, 2-way interleaved on the free dim (halves A/B alternate so
    each half's write-ack/semaphore round trip hides under the other half):
      1x FRAC10S per half (fused *0.1 + frac iter), then 30x FRAC10.
    Every link carries a self-semaphore (same-engine same-address RAW needs
    the write to land before the next read - verified racy without it).
    Per-iteration period is the model floor: max(2*t_instr, t_instr + L)
    where L = SBUF write-ack (60.4ns) + sem propagation (~35ns).
    A DVE drain then fences the finished chain for the PE.
  - PE: four fp32 [33k,32p,8q] matmuls sharing lhsT, one per rhs 8-column
    group, into PSUM partition ranges g*32.. (tile_position=(0,g*32)) so
    the result lands as [128,8] -- the shape kv_writeback's production
    dhi=128 layout needs. Same total PE row count as one [33,32,32]
    matmul. A PE drain fences the PSUM writes (skips the matmul sem's
    173ns modeled SBUF-access delay).
  - DVE [128,8] PSUM->SBUF copy (PSUM is not GPSIMD- or DMA-accessible),
    fenced by another DVE drain.
  - Pool (gpsimd) tail, everything expensive prepaid during the chain:
    memset ctx_idxs=0 and kv_writeback PREP (SWDGE descriptor generation,
    ~1.1us) both run at t~0, overlapped with the input DMA + DVE chain;
    after the copy fence, trigger_dma fires the pre-generated descriptors
    (no HWDGE 625ns gen + 650ns DGE delay on the critical path); a final
    wait on the DMA completion semaphore guarantees the writeback landed
    before the program ends; a last dma_reset drain resets all kernel
    semaphores + DMA state for NEFF re-execution (the end barrier's job).
  - The DRAM out tensor is [1,128,1,8]; host-side _unscramble() restores
    [32,32] row-major (verified element-exact on HW with a ramp).

Numerics are bit-exact IEEE RN fp32 vs the jax reference (verified on HW):
  frac iter: u=(v+1.5*2^23)-1.5*2^23 (=rne(v)); d=v-u (exact);
  out=(d+(d<0))*10 == (v-floor(v))*10 with a single fp32 rounding at *10.
"""
import numpy as np

import concourse.bacc as bacc
import concourse.bass as bass
import concourse.mybir as mybir
from concourse.bass_utils import run_bass_kernel_spmd
from concourse.dve_spec import Spec, Src0, C0, C1, C2, Zero
import concourse.dve_ops as dve_ops
from concourse.dve_ops import DveOp, OPS

F32 = mybir.dt.float32
I32 = mybir.dt.int32
N = 32
N_ITERS = 31
N_SPLIT = 2
N_CORES = 8
CMAGIC = float(np.float32(3.0 * 2.0**22))  # 1.5*2^23


def _frac_ref(in0, in1=None, s0=0.0, s1=0.0, imm2=0.0):
    u = ((in0 + np.float32(s0)).astype(np.float32) - np.float32(s0)).astype(np.float32)
    d = (in0 - u).astype(np.float32)
    return ((d + (d < 0).astype(np.float32)) * np.float32(s1)).astype(np.float32)


def _frac_s_ref(in0, in1=None, s0=0.0, s1=0.0, imm2=0.0):
    return _frac_ref((in0 * np.float32(imm2)).astype(np.float32), None, s0, s1)


def _register(name, spec, sha):
    for op in OPS:
        if op.name == name:
            return op
    op = DveOp(name, spec, subdim=False, uops_sha={"v3": sha})
    OPS.append(op)
    dve_ops.CUSTOM_DVE_SPECS[name] = op.spec
    dve_ops._SUB_OPCODE_FOR_NAME[name] = dve_ops._CUSTOM_DVE_ROW_BASE + len(OPS) - 1
    assert max(dve_ops._SUB_OPCODE_FOR_NAME.values()) < 0x20
    return op


def _register_ops():
    _u = (Src0 + C0) - C0
    _d = Src0 - _u
    frac10 = _register(
        "FRAC10", Spec(body=(_d + (_d < Zero)) * C1, reference=_frac_ref),
        "88c3f2aa3fac8098")
    _w = Src0 * C2
    _us = (_w + C0) - C0
    _ds = _w - _us
    frac10s = _register(
        "FRAC10S", Spec(body=(_ds + (_ds < Zero)) * C1, reference=_frac_s_ref),
        "d37aebb1b929ff2f")
    return frac10, frac10s


_NC_CACHE = {}


def _build(checked=False):
    """checked exists for API compat; both builds share one race-free sync
    graph (the DMA completion wait is native to the Pool tail), so the same
    program is CoreSim-validatable and production-fast."""
    if checked in _NC_CACHE:
        return _NC_CACHE[checked]
    frac10, frac10s = _register_ops()

    # The const-AP memsets + all-engine start barrier emitted by
    # Bass.__init__ serve tensors this kernel never reads (verified: walrus
    # flags them as reader-less); strip them to start the input DMA at t~0.
    _orig_barrier = bass.Bass.all_engine_barrier
    _orig_memset = bass.BassGpSimd.memset
    bass.Bass.all_engine_barrier = lambda self: None
    bass.BassGpSimd.memset = lambda self, ap, c: None
    try:
        nc = bacc.Bacc("TRN2", target_bir_lowering=False, debug=False)
    finally:
        bass.Bass.all_engine_barrier = _orig_barrier
        bass.BassGpSimd.memset = _orig_memset

    xp = nc.dram_tensor("xp", [N + 1, N], F32, kind="ExternalInput").ap()
    wp = nc.dram_tensor("wp", [N + 1, N], F32, kind="ExternalInput").ap()
    # kv_writeback production shape: d_head_inner = 128 partitions, dho=1
    # (the only dhi/dho split whose ucode/CoreSim semantics agree; verified
    # element-exact on HW with a ramp). Host-side _unscramble() restores
    # [32,32] row-major.
    out = nc.dram_tensor("out", [1, 128, 1, 8], F32, kind="ExternalOutput").ap()

    with (
        nc.sbuf_tensor("t", [N + 1, 2 * N], F32) as t,
        nc.sbuf_tensor("res", [128, 8], F32) as res,
        nc.sbuf_tensor("ctx_idx", [128, 1], I32) as ctx_idx,
        nc.psum_tensor("acc", [128, 8], F32) as acc,
        nc.semaphore("in_fence") as in_fence,
        nc.semaphore("w_fence") as w_fence,
        nc.semaphore("chain_fence") as chain_fence,
        nc.semaphore("dma_in_sem") as dma_in_sem,
        nc.semaphore("wb_sem") as wb_sem,
        nc.semaphore("dve_done") as dve_done,
        nc.semaphore("mm_done") as mm_done,
        nc.semaphore("copy_done") as copy_done,
        nc.semaphore("ms_sem") as ms_sem,
        nc.semaphore("prep_sem") as prep_sem,
        nc.semaphore("dma_out_sem") as dma_out_sem,
    ):
        # No nc.Block(): instructions are emitted straight into each
        # engine's main stream. This drops the per-engine entry branch
        # (50-96ns; the SP one delayed the input DMA and hence everything),
        # the body-exit branches, and the end-of-program all-engine barrier
        # (~290ns of gather+release after the last real work). The barrier's
        # other job -- resetting semaphore/DMA state for NEFF re-execution
        # -- is done by the Pool tail's dma_reset drain instead.

        # SP: x (+ones) first (it alone gates the chain), weights second.
        # The chain and the PE wait on the DMAs' own completion semaphores:
        # that carries SEM_PROP_DMA = 900ns of modeled signaling latency,
        # but it is the only input-ordering mechanism that is guaranteed on
        # real HW. (A drain-as-fence variant that started the chain ~1.5us
        # earlier in the model was INTERMITTENTLY WRONG on hardware -- the
        # SP drain does not reliably cover in-flight DMA transfers.)
        nc.sync.dma_start(t[:, 0:N], xp).then_inc(dma_in_sem, 16)
        nc.sync.dma_start(t[:, N : 2 * N], wp).then_inc(wb_sem, 16)
        nc.sync.drain()

        if True:
            W_ = N // N_SPLIT
            halves = [t[0:N, s * W_ : (s + 1) * W_] for s in range(N_SPLIT)]
            k = 0
            for s in range(N_SPLIT):
                ins = nc.vector._custom_dve(frac10s, out=halves[s], in0=halves[s],
                                            s0=CMAGIC, s1=10.0, imm2=0.1)
                ins._wait_ge(dma_in_sem, 16)
                ins.then_inc(dve_done, 1)
                k += 1
            for i in range(N_ITERS - 1):
                for s in range(N_SPLIT):
                    ins = nc.vector._custom_dve(frac10, out=halves[s], in0=halves[s],
                                                s0=CMAGIC, s1=10.0)
                    ins._wait_ge(dve_done, k - N_SPLIT + 1)
                    ins.then_inc(dve_done, 1)
                    k += 1
            # Fence the chain for the PE: the drain completes when the
            # DVE pipeline (and its SBUF writes) has flushed, without the
            # last chain instruction's 60ns write-ack + sem-prop detour.
            nc.vector.drain().then_inc(chain_fence, 1)
            # PSUM->SBUF copy must be DVE/Act: the BIR verifier rejects
            # GPSIMD PSUM accesses on trn2. One [128,8] copy (8 free elems)
            # is cheaper than the old [32,32] one (133ns vs 158ns engine).
            c = nc.vector.tensor_copy(res[:, :], acc[:, :])
            c._wait_ge(mm_done, 1)
            # Same fence idiom: DVE drain flushes the copy's SBUF writes,
            # so copy_done skips the 125ns write-ack the copy's own sem
            # update would carry in the cost model.
            nc.vector.drain().then_inc(copy_done, 1)

        if True:
            # Four [33k,32p,8q] matmuls into PSUM partition ranges g*32..:
            # the result lands as [128,8] so kv_writeback can use its
            # production dhi=128 shape. Same total PE row count as one
            # [33,32,32] matmul (fp32 cost = out free size * 4 cycles/row),
            # so the split is nearly free in PE time.
            nc.tensor.wait_ge(wb_sem, 16)
            for g in range(4):
                ins = nc.tensor.matmul(
                    acc[g * N : (g + 1) * N, :],
                    t[:, 0:N],
                    t[:, N + g * 8 : N + (g + 1) * 8],
                    start=True, stop=True,
                    tile_position=(0, g * N))
                if g == 0:
                    ins._wait_ge(chain_fence, 1)
            # PE pipeline fence instead of then_inc on the last matmul:
            # the drain completes when the PE pipeline has flushed (real-HW
            # write-visibility fence), without the cost model's 146ns
            # min_engine_delay on the matmul's own sem update.
            nc.tensor.drain().then_inc(mm_done, 1)

        if True:
            g = nc.gpsimd
            # All of this except the trigger runs at t~0, overlapped
            # with the input DMA + DVE chain.
            g.memset(ctx_idx[:, :], 0).then_inc(ms_sem, 1)
            in4 = res[:, :].rearrange("p (a c b) -> p a c b", a=1, c=1, b=8)
            p = g.kv_writeback(out, in4, ctx_idx[:, :],
                               prepare_only=True, sem=dma_out_sem)
            p._wait_ge(ms_sem, 1)
            p.then_inc(prep_sem, 1)
            g.wait_ge(prep_sem, 1)
            trig = g.trigger_dma(count=1)
            trig._wait_ge(copy_done, 1)
            # Writeback-landed guarantee before the program ends.
            g.wait_ge(dma_out_sem, 16)
            # Rerun safety without the end barrier: one drain that resets
            # all kernel semaphores + associated DMA state. Every consumer
            # of these sems has finished by the time the dma_out wait above
            # passes (Pool is the last engine running).
            sem_nums = [s.num for s in (in_fence, w_fence, chain_fence,
                                        dma_in_sem, wb_sem, dve_done, mm_done,
                                        copy_done, ms_sem, prep_sem,
                                        dma_out_sem)]
            g.dma_reset(range(min(sem_nums), max(sem_nums) + 1))

    nc.compile()
    _NC_CACHE[checked] = nc
    return nc


def _pack(x, W, b):
    xp = np.empty((N + 1, N), dtype=np.float32)
    xp[0:N, :] = x.T
    xp[N, :] = 1.0
    wp = np.empty((N + 1, N), dtype=np.float32)
    wp[0:N, :] = W.T
    wp[N, :] = np.float32(32.0) * b
    return xp, wp


def _unscramble(arr: np.ndarray) -> np.ndarray:
    """[1,128,1,8] kv_writeback layout -> [32,32] row-major output.
    Partition h = g*32+p of the PSUM/SBUF result holds out[p, g*8:(g+1)*8]
    (matmul g computed rhs columns g*8..)."""
    return (np.asarray(arr, dtype=np.float32).reshape(4, N, 8)
            .transpose(1, 0, 2).reshape(N, N))


def kernel(x: np.ndarray, W: np.ndarray, b: np.ndarray) -> np.ndarray:
    x = np.asarray(x, dtype=np.float32)
    W = np.asarray(W, dtype=np.float32)
    b = np.asarray(b, dtype=np.float32)
    nc = _build()
    xp, wp = _pack(x, W, b)
    in_map = {"xp": xp, "wp": wp}
    res = run_bass_kernel_spmd(nc, [in_map] * N_CORES, core_ids=list(range(N_CORES)))
    return _unscramble(res.results[0]["out"])
